# revision 39
# baseline (speedup 1.0000x reference)
"""Trainium2 Bass kernel for LoFTR-style encoder layer (sparse attention + convs).

Sharding: pure data-parallel over batch B=8 -> 8 NeuronCores (one batch
element per core). BN statistics are all-reduced across cores.

Schedule: conv1's row-tiles are interleaved with the vector-bound
attention phase (f-only tiles have no message dependency; later tiles
follow one message-tile behind) so the PE never starves; BN uses a
single all-reduce per layer (launch skew between cores is absorbed
exactly once) preceded by two dummy warm-up collectives that absorb the
~20us CC cold-start during the attention window; BN1 is folded into
conv2 (weights scaled by scl1, bias from host-precomputed tap-summed
weights, padding halo written as -sh1/scl1 so zero-padding is
reproduced exactly); output is stored bf16 and converted on host.

Device layout is channel-major ([C, spatial]); host does the (free)
transposes / weight reordering when staging inputs, and transposes the
per-core outputs back.
"""

import os
import sys

import numpy as np

for _p in ("/opt/trn_rl_repo", os.path.expanduser("~/.axon_site/_ro/trn_rl_repo")):
    if os.path.isdir(_p) and _p not in sys.path:
        sys.path.insert(0, _p)

import ml_dtypes

import concourse.bass as bass
import concourse.mybir as mybir
import concourse.tile as tile
from concourse import bacc
from concourse.bass_utils import run_bass_kernel_spmd

F32 = mybir.dt.float32
BF16 = mybir.dt.bfloat16
AF = mybir.ActivationFunctionType
ALU = mybir.AluOpType

NCORES = 8
H = W = 80
HW = H * W          # 6400
D = 256
NI = 3200           # inside positions (image rows 0..39)
NHEAD = 8
PW = W + 2          # 82 padded width
BN_EPS = 1e-5
BN_N = float(NCORES * HW)

# conv row-tiling: 5 output rows per psum tile -> N = 5*82 = 410 <= 512
RT = 5
NRT = H // RT       # 16
NT = RT * PW        # 410

LAST_EXEC_NS = None
LAST_MEAN_EXEC_NS = None

_cache = {}


def _bd(ap3):
    return ap3.rearrange("p a b -> p (a b)")


def _r3(ap2, a):
    return ap2.rearrange("p (a b) -> p a b", a=a)


def build_nc():
    nc = bacc.Bacc(
        "TRN2", target_bir_lowering=False, debug=False, num_devices=NCORES
    )

    ft_d = nc.dram_tensor("ft", [D, HW], F32, kind="ExternalInput")
    wqt_d = nc.dram_tensor("wqt", [128, 2, D], BF16, kind="ExternalInput")
    wkt_d = nc.dram_tensor("wkt", [128, 2, D], BF16, kind="ExternalInput")
    wvt_d = nc.dram_tensor("wvt", [128, 2, D], BF16, kind="ExternalInput")
    c1w_d = nc.dram_tensor("c1w", [128, 36, D], BF16, kind="ExternalInput")
    c2w_d = nc.dram_tensor("c2w", [128, 18, D], BF16, kind="ExternalInput")
    ws2_d = nc.dram_tensor("ws2", [128, 2, D], BF16, kind="ExternalInput")
    bn1g_d = nc.dram_tensor("bn1g", [D, 1], F32, kind="ExternalInput")
    bn1b_d = nc.dram_tensor("bn1b", [D, 1], F32, kind="ExternalInput")
    bn2g_d = nc.dram_tensor("bn2g", [D, 1], F32, kind="ExternalInput")
    bn2b_d = nc.dram_tensor("bn2b", [D, 1], F32, kind="ExternalInput")
    mblk_d = nc.dram_tensor("mblk", [8, 256], BF16, kind="ExternalInput")
    out_d = nc.dram_tensor("out_t", [D, HW], BF16, kind="ExternalOutput")

    groups = [list(range(NCORES))]

    with tile.TileContext(nc) as tc:
        with (
            tc.tile_pool(name="pers", bufs=1) as pers,
            tc.tile_pool(name="bigp", bufs=2) as bigp,
            tc.tile_pool(name="ftio", bufs=2) as ftio,
            tc.tile_pool(name="qtp", bufs=4) as qtp,
            tc.tile_pool(name="scr", bufs=3) as scr,
            tc.tile_pool(name="small", bufs=1) as small,
            tc.tile_pool(name="fin", bufs=4) as fin,
            tc.tile_pool(name="psA", bufs=4, space="PSUM") as psA,
            tc.tile_pool(name="psS", bufs=2, space="PSUM") as psS,
            tc.tile_pool(name="psC", bufs=2, space="PSUM") as psC,
            tc.tile_pool(name="dram", bufs=1, space="DRAM") as dramp,
        ):
            # ---------------- weights / consts (DMA: proj first, ft next) ---
            wqt = pers.tile([128, 2, D], BF16, tag="wqt", name="wqt")
            wkt = pers.tile([128, 2, D], BF16, tag="wkt", name="wkt")
            wvt = pers.tile([128, 2, D], BF16, tag="wvt", name="wvt")
            nc.sync.dma_start(wkt[:, :, :], wkt_d[:, :, :])
            nc.sync.dma_start(wvt[:, :, :], wvt_d[:, :, :])
            nc.sync.dma_start(wqt[:, :, :], wqt_d[:, :, :])

            # ft segments: (s, m) order so K/V projections can start early
            ftb = [
                pers.tile([128, HW], BF16, tag=f"ftb{m}", name=f"ftb{m}")
                for m in range(2)
            ]
            ip1 = [
                pers.tile([128, 84, PW], BF16, tag=f"ip1_{c}", name=f"ip1_{c}")
                for c in range(4)
            ]
            ip1f = [_bd(ip1[c][:, :, :]) for c in range(4)]

            # first-256-column slivers let the first K/V matmuls start
            # ~8us earlier than the full 1600-col segment loads allow
            ft_sl = {}
            for m in range(2):
                slv = ftio.tile([128, 256], F32, tag="sliv", name="sliv")
                nc.sync.dma_start(slv[:, :], ft_d[m * 128 : (m + 1) * 128, 0:256])
                ft_sl[m] = slv
            ft_stage = {}
            for s in range(4):
                for m in range(2):
                    lo = 256 if s == 0 else 0
                    ftt = ftio.tile([128, 1600 - lo], F32, tag="ftio", name="ftio")
                    eng = nc.sync if m == 0 else nc.gpsimd
                    eng.dma_start(
                        ftt[:, :],
                        ft_d[m * 128 : (m + 1) * 128,
                             s * 1600 + lo : (s + 1) * 1600],
                    )
                    ft_stage[(s, m)] = ftt

            c1w = pers.tile([128, 36, D], BF16, tag="c1w", name="c1w")
            c2w = pers.tile([128, 18, D], BF16, tag="c2w", name="c2w")
            ws2 = pers.tile([128, 2, D], BF16, tag="ws2", name="ws2")
            maskblk = pers.tile([8, 256], BF16, tag="maskblk", name="maskblk")
            nc.sync.dma_start(c1w[:, :, :], c1w_d[:, :, :])
            nc.sync.dma_start(c2w[:, :, :], c2w_d[:, :, :])
            nc.sync.dma_start(ws2[:, :, :], ws2_d[:, :, :])
            nc.sync.dma_start(maskblk[:, :], mblk_d[:, :])

            # warm up the CC cores with two dummy all-reduces: the first two
            # collective executions pay a ~20us cold-start which would
            # otherwise land in the BN1 bubble; here it overlaps attention.
            wrm = small.tile([8, 2], F32, tag="wrm", name="wrm")
            nc.vector.memset(wrm[:, :], 0.0)
            for w in range(2):
                win = dramp.tile([8, 2], F32, tag=f"win{w}", name=f"win{w}")
                wout = dramp.tile([8, 2], F32, tag=f"wout{w}", name=f"wout{w}")
                nc.sync.dma_start(win[:, :], wrm[:, :])
                nc.gpsimd.collective_compute(
                    "AllReduce", ALU.add, replica_groups=groups,
                    ins=[win[:, :].opt()], outs=[wout[:, :].opt()],
                )

            eps_t = small.tile([128, 1], F32, tag="eps_t", name="eps_t")
            nc.vector.memset(eps_t[:, :], BN_EPS)
            ones_t = small.tile([128, 164], F32, tag="ones_t", name="ones_t")
            nc.vector.memset(ones_t[:, :], 1.0)
            g1 = small.tile([128, 2], F32, tag="g1", name="g1")
            b1 = small.tile([128, 2], F32, tag="b1", name="b1")
            g2 = small.tile([128, 2], F32, tag="g2", name="g2")
            b2 = small.tile([128, 2], F32, tag="b2", name="b2")
            for o in range(2):
                sl = slice(o * 128, (o + 1) * 128)
                nc.sync.dma_start(g1[:, o : o + 1], bn1g_d[sl, :])
                nc.sync.dma_start(b1[:, o : o + 1], bn1b_d[sl, :])
                nc.sync.dma_start(g2[:, o : o + 1], bn2g_d[sl, :])
                nc.sync.dma_start(b2[:, o : o + 1], bn2b_d[sl, :])

            # ---------------- ip1 halo zeroing (targeted, not full tiles) ---
            # tile row r+2 == image row r; taps read tile rows 1..82 and the
            # 410-window spills touch rows 0 and 83 in discarded columns.
            for c in range(2):  # f-chunks
                nc.vector.memset(ip1[c][:, 0:2, :], 0.0)
                nc.vector.memset(ip1[c][:, 82:84, :], 0.0)
                nc.vector.memset(ip1[c][:, 2:82, 0:1], 0.0)
                nc.vector.memset(ip1[c][:, 2:82, 81:82], 0.0)
            for c in range(2, 4):  # t-chunks: zero band above message rows
                nc.vector.memset(ip1[c][:, 34:42, :], 0.0)
                nc.vector.memset(ip1[c][:, 82:84, :], 0.0)
                nc.vector.memset(ip1[c][:, 42:82, 0:1], 0.0)
                nc.vector.memset(ip1[c][:, 42:82, 81:82], 0.0)

            # ---------------- per-segment cast + ip1 interior + K/V proj ----
            ke = bigp.tile([128, 25, D], BF16, tag="big", name="ke")
            # ve layout [ones, v0..v255, ones]: per half m the 129 columns
            # [m*129 : m*129+129) are contiguous = [ones|v_m] or [v_m|ones]
            ve = bigp.tile([128, 25, D + 2], BF16, tag="big", name="ve")
            nc.vector.memset(ve[:, :, 0:1], 1.0)
            nc.vector.memset(ve[:, :, 257:258], 1.0)

            def kv_tile(i):
                ps = psA.tile([128, D], F32, tag="psA", name="psA")
                for ki in range(2):
                    nc.tensor.matmul(
                        ps[:, :],
                        ftb[ki][:, i * 128 : (i + 1) * 128],
                        wkt[:, ki, :],
                        start=(ki == 0),
                        stop=(ki == 1),
                    )
                # elu(x)+1 = relu(x) + exp(min(x,0))
                sm = scr.tile([128, 400], F32, tag="scr", name="sm")
                se = scr.tile([128, 400], F32, tag="scr", name="se")
                nc.vector.tensor_scalar_min(sm[:, :D], ps[:, :], 0.0)
                nc.scalar.activation(se[:, :D], sm[:, :D], AF.Exp)
                nc.vector.scalar_tensor_tensor(
                    ke[:, i, :], ps[:, :], 0.0, se[:, :D], ALU.max, ALU.add
                )
                ps2 = psA.tile([128, D], F32, tag="psA", name="psA")
                for ki in range(2):
                    nc.tensor.matmul(
                        ps2[:, :],
                        ftb[ki][:, i * 128 : (i + 1) * 128],
                        wvt[:, ki, :],
                        start=(ki == 0),
                        stop=(ki == 1),
                    )
                nc.vector.tensor_copy(ve[:, i, 1:257], ps2[:, :])

            def stage_seg(s, m):
                ftt = ft_stage[(s, m)]
                lo = 256 if s == 0 else 0
                nc.scalar.copy(
                    ftb[m][:, s * 1600 + lo : (s + 1) * 1600], ftt[:, :]
                )
                fseg = ftb[m][:, s * 1600 : (s + 1) * 1600]
                nc.vector.tensor_copy(
                    ip1[m][:, 2 + 20 * s : 22 + 20 * s, 1:81], _r3(fseg, 20)
                )

            for m in range(2):
                nc.scalar.copy(ftb[m][:, 0:256], ft_sl[m][:, :])
            kv_tile(0)
            kv_tile(1)
            for s in range(2):  # inside segments
                for m in range(2):
                    stage_seg(s, m)
                for i in range(12 * s + 2 * (1 - s), 12 * s + 12 + s):
                    kv_tile(i)  # s=0: 2..11, s=1: 12..24

            for s in range(2, 4):  # outside segments
                for m in range(2):
                    stage_seg(s, m)

            # ---------------- conv1 helpers ----------------
            y1p = [
                pers.tile([128, 84, PW], BF16, tag=f"y1p_{o}", name=f"y1p_{o}")
                for o in range(2)
            ]
            for o in range(2):  # spill-read guard rows (discarded columns)
                nc.vector.memset(y1p[o][:, 0:1, :], 0.0)
                nc.vector.memset(y1p[o][:, 83:84, :], 0.0)
            stats1 = small.tile([128, 4 * NRT], F32, tag="stats1", name="stats1")

            def conv1_tile(j):
                r0 = RT * j
                taps = []
                for c in range(4):
                    for ky in range(3):
                        if c >= 2 and r0 + ky + 4 < 41:
                            continue  # t-channel rows all zero
                        for kx in range(3):
                            taps.append((c, ky, kx))
                for o in range(2):
                    ps = psC.tile([128, NT], F32, tag="psC", name="psC")
                    for idx, (c, ky, kx) in enumerate(taps):
                        s = (r0 + ky + 1) * PW + kx - 1
                        nc.tensor.matmul(
                            ps[:, :],
                            c1w[:, (ky * 3 + kx) * 4 + c, o * 128 : (o + 1) * 128],
                            ip1f[c][:, s : s + NT],
                            start=(idx == 0),
                            stop=(idx == len(taps) - 1),
                        )
                    val = _r3(ps[:, :], RT)[:, :, 1:81]
                    nc.scalar.copy(
                        y1p[o][:, 2 + r0 : 7 + r0, 1:81], val
                    )
                    nc.vector.tensor_reduce(
                        stats1[:, (o * 2) * NRT + j : (o * 2) * NRT + j + 1],
                        val, mybir.AxisListType.XY, ALU.add,
                    )
                    sq = scr.tile([128, 400], F32, tag="scr", name="sq")
                    nc.scalar.activation(
                        _r3(sq[:, :], RT), val, AF.Square,
                        accum_out=stats1[:, (o * 2 + 1) * NRT + j :
                                         (o * 2 + 1) * NRT + j + 1],
                    )

            # f-only conv1 tiles 0,1 ahead of the KV->bd accumulation:
            # the bd chain waits on per-i scalar ve copies, so give the PE
            # filler work to absorb that lag.
            conv1_tile(0)
            conv1_tile(1)

            # ---------------- KV + Ksum -> block-diag BD ----------------
            bd = [
                pers.tile([128, 136], BF16, tag=f"bd{m}", name=f"bd{m}")
                for m in range(2)
            ]
            for m in range(2):
                psm = psA.tile([128, 129], F32, tag="psA", name="psA")
                for i in range(25):
                    nc.tensor.matmul(
                        psm[:, :],
                        ke[:, i, m * 128 : (m + 1) * 128],
                        ve[:, i, m * 129 : m * 129 + 129],
                        start=(i == 0),
                        stop=(i == 24),
                    )
                kcol = 0 if m == 0 else 128
                voff = 1 - m
                nc.vector.memset(bd[m][:, :], 0.0)
                for hh in range(4):
                    h = m * 4 + hh
                    lh = hh * 32
                    nc.vector.tensor_copy(
                        bd[m][lh : lh + 32, lh : lh + 32],
                        psm[lh : lh + 32, voff + lh : voff + lh + 32],
                    )
                    nc.vector.tensor_copy(
                        bd[m][lh : lh + 32, 128 + h : 129 + h],
                        psm[lh : lh + 32, kcol : kcol + 1],
                    )

            # ---------- Q/message pipeline interleaved with f-only conv1 ----
            def qproj(j):
                lsl = slice(NI + j * 400, NI + (j + 1) * 400)
                qt = [
                    qtp.tile([128, 400], BF16, tag="qteT", name=f"qt{m}")
                    for m in range(2)
                ]
                qps = []
                for m in range(2):
                    ps = psA.tile([128, 400], F32, tag="psA", name="psA")
                    for ki in range(2):
                        nc.tensor.matmul(
                            ps[:, :],
                            wqt[:, ki, m * 128 : (m + 1) * 128],
                            ftb[ki][:, lsl],
                            start=(ki == 0),
                            stop=(ki == 1),
                        )
                    qps.append(ps)
                return qt, qps

            def qelu(qt, qps):
                for m in range(2):
                    sm = scr.tile([128, 400], F32, tag="scr", name="smq")
                    se = scr.tile([128, 400], F32, tag="scr", name="seq")
                    nc.vector.tensor_scalar_min(sm[:, :], qps[m][:, :], 0.0)
                    nc.scalar.activation(se[:, :], sm[:, :], AF.Exp)
                    nc.vector.scalar_tensor_tensor(
                        qt[m][:, :], qps[m][:, :], 0.0, se[:, :], ALU.max, ALU.add
                    )

            def qmessage(j, qt):
                pss = psS.tile([8, 400], F32, tag="psS", name="psS")
                for ki in range(2):
                    nc.tensor.matmul(
                        pss[:, :],
                        bd[ki][:, 128:136],
                        qt[ki][:, :],
                        start=(ki == 0),
                        stop=(ki == 1),
                    )
                # S >> eps (S >= ~1e2), so 1/(S+eps) == 1/S in fp32
                rsf = scr.tile([128, 400], F32, tag="scr", name="rsf")
                rs = scr.tile([128, 400], BF16, tag="scr", name="rs")
                nc.vector.reciprocal_approx_fast(rsf[:8, :], pss[:, :])
                nc.scalar.copy(rs[:8, :], rsf[:8, :])

                for m in range(2):
                    psg = psA.tile([128, 400], F32, tag="psA", name="psA")
                    nc.tensor.matmul(
                        psg[:, :], bd[m][:, 0:128], qt[m][:, :],
                        start=True, stop=True,
                    )
                    pre = psS.tile([128, 400], F32, tag="psS", name="psS")
                    nc.tensor.matmul(
                        pre[:, :], maskblk[:, m * 128 : (m + 1) * 128], rs[:8, :]
                    )
                    preb = scr.tile([128, 400], BF16, tag="scr", name="preb")
                    nc.scalar.copy(preb[:, :], pre[:, :])
                    # l-tile j = image rows 40+5j..44+5j -> tile rows 42+5j..
                    nc.vector.tensor_tensor(
                        ip1[2 + m][:, 42 + 5 * j : 47 + 5 * j, 1:81],
                        _r3(psg[:, :], RT),
                        _r3(preb[:, :], RT),
                        ALU.mult,
                    )

            # software pipeline: Qproj j+1 is issued before the dependent
            # attention tail of j; f-only conv1 tiles fill the PE while the
            # vector engine runs elu/Z.
            qt_c, qps_c = qproj(0)
            for j in range(8):
                qelu(qt_c, qps_c)
                if j <= 4:
                    conv1_tile(j + 2)   # f-only tiles (t rows all zero)
                nxt = qproj(j + 1) if j < 7 else None
                qmessage(j, qt_c)
                if j >= 1:
                    # tile j+6 needs message j-1 (just written): keeps the PE
                    # queue deep so the elu chain latency never starves it
                    conv1_tile(j + 6)
                if nxt is not None:
                    qt_c, qps_c = nxt

            # pre-warm the sqrt activation table now that the last Exp is
            # issued: the 1.28us table switch hides under conv1 instead of
            # landing in the BN1 critical path.
            sqwrm = small.tile([128, 1], F32, tag="sqwrm", name="sqwrm")
            nc.scalar.activation(sqwrm[:, :], eps_t[:, :], AF.Sqrt)

            # two-chunk BN all-reduce: chunk A (tiles 0..CHA-1) is launched
            # while the PE is still crunching the remaining tiles; only the
            # small chunk-B collective latency is exposed.
            def ar_chunk(stats, lo, hi, tag):
                sv = stats[:, :].rearrange("p (k j) -> p k j", j=NRT)
                bnst = small.tile([128, 4], F32, tag=f"bnst{tag}", name=f"bnst{tag}")
                arin = dramp.tile([128, 4], F32, tag=f"arin{tag}", name=f"arin{tag}")
                arout = dramp.tile([128, 4], F32, tag=f"arout{tag}", name=f"arout{tag}")
                nc.vector.tensor_reduce(
                    bnst[:, :], sv[:, :, lo:hi], mybir.AxisListType.X, ALU.add
                )
                return bnst, arin, arout

            def ar_launch(bnst, arin, arout):
                nc.gpsimd.dma_start(arin[:, :], bnst[:, :])
                nc.gpsimd.collective_compute(
                    "AllReduce", ALU.add, replica_groups=groups,
                    ins=[arin[:, :].opt()], outs=[arout[:, :].opt()],
                )

            def ar_fetch(arout, tag):
                g = small.tile([128, 4], F32, tag=f"gst{tag}", name=f"gst{tag}")
                nc.gpsimd.dma_start(g[:, :], arout[:, :])
                return g

            # a single all-reduce per BN: launch skew between cores is
            # absorbed exactly once (chunked/split all-reduces each absorb
            # the full skew again since their triggers are independent).
            for j in range(14, NRT):
                conv1_tile(j)
            ar1 = ar_chunk(stats1, 0, NRT, "1")
            ar_launch(*ar1)
            gst1 = ar_fetch(ar1[2], "1")

            def bn_coeffs(gst, gg, bb, tag):
                nm = small.tile([128, 2], F32, tag=f"nm{tag}", name=f"nm{tag}")
                ex2 = small.tile([128, 2], F32, tag=f"ex2{tag}", name=f"ex2{tag}")
                var = small.tile([128, 2], F32, tag=f"var{tag}", name=f"var{tag}")
                sd = small.tile([128, 2], F32, tag=f"sd{tag}", name=f"sd{tag}")
                rsd = small.tile([128, 2], F32, tag=f"rsd{tag}", name=f"rsd{tag}")
                scl = small.tile([128, 2], F32, tag=f"scl{tag}", name=f"scl{tag}")
                sh = small.tile([128, 2], F32, tag=f"sh{tag}", name=f"sh{tag}")
                gv = gst[:, :].rearrange("p (o k) -> p k o", k=2)
                nc.vector.tensor_scalar_mul(nm[:, :], gv[:, 0, :], -1.0 / BN_N)
                nc.vector.tensor_scalar_mul(ex2[:, :], gv[:, 1, :], 1.0 / BN_N)
                # var_neg = m^2 - E[x^2];  sd = sqrt(-var_neg + eps)
                nc.vector.tensor_tensor(var[:, :], nm[:, :], nm[:, :], ALU.mult)
                nc.vector.tensor_tensor(
                    var[:, :], var[:, :], ex2[:, :], ALU.subtract
                )
                nc.scalar.activation(
                    sd[:, :], var[:, :], AF.Sqrt, bias=eps_t[:, 0:1], scale=-1.0
                )
                nc.vector.reciprocal(rsd[:, :], sd[:, :])
                nc.vector.tensor_tensor(scl[:, :], rsd[:, :], gg[:, :], ALU.mult)
                nc.vector.tensor_tensor(sh[:, :], nm[:, :], scl[:, :], ALU.mult)
                nc.vector.tensor_tensor(sh[:, :], sh[:, :], bb[:, :], ALU.add)
                return scl, sh

            scl1, sh1 = bn_coeffs(gst1, g1, b1, "1")

            # fold BN1 into conv2: w2' = w2 * scl1[c]; halo = -sh1/scl1 so
            # zero-padding maps to BN-output zero; bias2[o] = sum_{c,k} w2*sh1
            c2wv = c2w[:, :, :].rearrange("p (t c) o -> p t c o", c=2)
            for ck in range(2):
                nc.vector.tensor_scalar(
                    c2wv[:, :, ck, :], c2wv[:, :, ck, :],
                    scl1[:, ck : ck + 1], None, ALU.mult,
                )
            hv1 = small.tile([128, 2], F32, tag="hv1", name="hv1")
            rscl = small.tile([128, 2], F32, tag="rscl", name="rscl")
            nc.vector.reciprocal(rscl[:, :], scl1[:, :])
            nc.vector.scalar_tensor_tensor(
                hv1[:, :], sh1[:, :], -1.0, rscl[:, :], ALU.mult, ALU.mult
            )
            for o in range(2):
                hvo = hv1[:, o : o + 1]
                nc.vector.tensor_scalar(
                    y1p[o][:, 1:2, :], _r3(ones_t[:, 0:82], 1), hvo, None, ALU.mult
                )
                nc.vector.tensor_scalar(
                    y1p[o][:, 82:83, :], _r3(ones_t[:, 0:82], 1), hvo, None, ALU.mult
                )
                nc.vector.tensor_scalar(
                    y1p[o][:, 2:82, 0:1], _r3(ones_t[:, 0:80], 80), hvo, None,
                    ALU.mult,
                )
                nc.vector.tensor_scalar(
                    y1p[o][:, 2:82, 81:82], _r3(ones_t[:, 0:80], 80), hvo, None,
                    ALU.mult,
                )

            bias2 = small.tile([128, 2], F32, tag="bias2", name="bias2")
            sh1b = small.tile([128, 2], BF16, tag="sh1b", name="sh1b")
            nc.scalar.copy(sh1b[:, :], sh1[:, :])
            for o in range(2):
                psb = psC.tile([128, NT], F32, tag="psC", name="psC")
                for ck in range(2):
                    nc.tensor.matmul(
                        psb[:, 0:1],
                        ws2[:, ck, o * 128 : (o + 1) * 128],
                        sh1b[:, ck : ck + 1],
                        start=(ck == 0),
                        stop=(ck == 1),
                    )
                nc.scalar.copy(bias2[:, o : o + 1], psb[:, 0:1])

            # ---------------- conv2 (+ stats) ----------------
            y2 = [
                bigp.tile([128, HW], BF16, tag="big", name=f"y2_{o}")
                for o in range(2)
            ]
            stats2 = small.tile([128, 4 * NRT], F32, tag="stats2", name="stats2")
            y1pf = [_bd(y1p[c][:, :, :]) for c in range(2)]
            for j in range(NRT):
                r0 = RT * j
                for o in range(2):
                    ps = psC.tile([128, NT], F32, tag="psC", name="psC")
                    idx = 0
                    for c in range(2):
                        for ky in range(3):
                            for kx in range(3):
                                s = (r0 + ky + 1) * PW + kx - 1
                                nc.tensor.matmul(
                                    ps[:, :],
                                    c2w[:, (ky * 3 + kx) * 2 + c,
                                        o * 128 : (o + 1) * 128],
                                    y1pf[c][:, s : s + NT],
                                    start=(idx == 0),
                                    stop=(idx == 17),
                                )
                                idx += 1
                    val = _r3(ps[:, :], RT)[:, :, 1:81]
                    # y2 = conv2(BN1(y1)) = ps + bias2 (scalar adds the bias)
                    nc.scalar.activation(
                        _r3(y2[o][:, j * 400 : (j + 1) * 400], RT), val,
                        AF.Identity, bias=bias2[:, o : o + 1],
                    )
                    nc.vector.tensor_reduce(
                        stats2[:, (o * 2) * NRT + j : (o * 2) * NRT + j + 1],
                        val, mybir.AxisListType.XY, ALU.add,
                    )
                    sq = scr.tile([128, 400], F32, tag="scr", name="sq2")
                    nc.scalar.activation(
                        _r3(sq[:, :], RT), val, AF.Square,
                        bias=bias2[:, o : o + 1],
                        accum_out=stats2[:, (o * 2 + 1) * NRT + j :
                                         (o * 2 + 1) * NRT + j + 1],
                    )

            # ---------------- BN2 allreduce ----------------
            ar2 = ar_chunk(stats2, 0, NRT, "2")
            # reduce() summed raw psum values; the true sums need +HW*bias2
            # exactly once per core (the sq stats were already biased).
            bnst2 = ar2[0]
            for o in range(2):
                nc.vector.scalar_tensor_tensor(
                    bnst2[:, 2 * o : 2 * o + 1], bias2[:, o : o + 1],
                    float(HW), bnst2[:, 2 * o : 2 * o + 1], ALU.mult, ALU.add
                )
            ar_launch(*ar2)
            gst2 = ar_fetch(ar2[2], "2")
            scl2, sh2 = bn_coeffs(gst2, g2, b2, "2")

            # ---------------- BN2 + residual + store (bf16, all-DVE) -------
            for o in range(2):
                for j in range(8):
                    fsl = slice(800 * j, 800 * (j + 1))
                    tmp = fin.tile([128, 800], BF16, tag="tmp", name="tmp")
                    nc.scalar.activation(
                        tmp[:, :], y2[o][:, fsl], AF.Identity,
                        bias=sh2[:, o : o + 1], scale=scl2[:, o : o + 1],
                    )
                    ost = fin.tile([128, 800], BF16, tag="ost", name="ost")
                    nc.vector.tensor_tensor(
                        ost[:, :], tmp[:, :], ftb[o][:, fsl], ALU.add
                    )
                    oeng = nc.sync if j % 2 == 0 else nc.gpsimd
                    oeng.dma_start(out_d[o * 128 : (o + 1) * 128, fsl], ost[:, :])

    nc.compile()
    return nc


def _mblk():
    mb = np.zeros((8, 256), np.float32)
    for h in range(8):
        mb[h, h * 32 : (h + 1) * 32] = 1.0
    return mb.astype(ml_dtypes.bfloat16)


def _prep_inputs(feat0, zone_mask, w_q, w_k, w_v, conv1_w, bn1_g, bn1_b,
                 conv2_w, bn2_g, bn2_b, num_inside):
    B = feat0.shape[0]
    pos = np.asarray(zone_mask[:, :, 0])
    order = np.argsort(~pos, axis=1, kind="stable")
    assert np.array_equal(
        order[:, :num_inside],
        np.broadcast_to(np.arange(num_inside), (B, num_inside)),
    ), "kernel assumes inside positions are the first num_inside rows"
    assert num_inside == NI

    bf = ml_dtypes.bfloat16
    f32 = np.float32

    def wt(w):  # [dout, din] -> [128, 2, dout]: [p, ki, o] = w[o, ki*128+p]
        return np.ascontiguousarray(
            w.T.reshape(2, 128, D).transpose(1, 0, 2)
        ).astype(bf)

    def cw(w, nchunk):  # [O, I, 3, 3] -> [128, 9*nchunk, O]
        o_, i_, _, _ = w.shape
        r = w.transpose(2, 3, 1, 0).reshape(9, nchunk, 128, o_)
        return np.ascontiguousarray(
            r.transpose(2, 0, 1, 3).reshape(128, 9 * nchunk, o_)
        ).astype(bf)

    c2 = np.asarray(conv2_w, f32)
    # tap-summed conv2 weights for the folded-BN bias: [128, chunk, O]
    ws2 = np.ascontiguousarray(
        c2.sum(axis=(2, 3)).T.reshape(2, 128, D).transpose(1, 0, 2)
    ).astype(bf)

    common = {
        "wqt": wt(np.asarray(w_q, f32)),
        "wkt": wt(np.asarray(w_k, f32)),
        "wvt": wt(np.asarray(w_v, f32)),
        "c1w": cw(np.asarray(conv1_w, f32), 4),
        "c2w": cw(c2, 2),
        "ws2": ws2,
        "bn1g": np.asarray(bn1_g, f32).reshape(D, 1),
        "bn1b": np.asarray(bn1_b, f32).reshape(D, 1),
        "bn2g": np.asarray(bn2_g, f32).reshape(D, 1),
        "bn2b": np.asarray(bn2_b, f32).reshape(D, 1),
        "mblk": _mblk(),
    }
    in_maps = []
    for b in range(NCORES):
        m = dict(common)
        m["ft"] = np.ascontiguousarray(np.asarray(feat0[b], f32).T)
        in_maps.append(m)
    return in_maps


def kernel(feat0, zone_mask, w_q, w_k, w_v, conv1_w, bn1_g, bn1_b,
           conv2_w, bn2_g, bn2_b, H=80, W=80, B=8, D=256, num_inside=3200,
           **_ignored):
    global LAST_EXEC_NS, LAST_MEAN_EXEC_NS
    if "nc" not in _cache:
        _cache["nc"] = build_nc()
    nc = _cache["nc"]

    in_maps = _prep_inputs(feat0, zone_mask, w_q, w_k, w_v, conv1_w, bn1_g,
                           bn1_b, conv2_w, bn2_g, bn2_b, int(num_inside))
    trace = os.environ.get("KERNEL_TRACE", "0") == "1"
    res = run_bass_kernel_spmd(nc, in_maps, list(range(NCORES)), trace=trace)
    LAST_EXEC_NS = res.exec_time_ns
    LAST_MEAN_EXEC_NS = res.mean_exec_time_ns
    out = np.empty((NCORES, HW, 256), np.float32)
    for b in range(NCORES):
        out[b] = np.asarray(res.results[b]["out_t"], np.float32).T
    return out


# revision 40
# speedup vs baseline: 1.0393x; 1.0393x over previous
"""Trainium2 Bass kernel for LoFTR-style encoder layer (sparse attention + convs).

Sharding: pure data-parallel over batch B=8 -> 8 NeuronCores (one batch
element per core). BN statistics are all-reduced across cores.

Schedule: conv1's row-tiles are interleaved with the vector-bound
attention phase (f-only tiles have no message dependency; later tiles
follow one message-tile behind) so the PE never starves; BN uses a
single all-reduce per layer (launch skew between cores is absorbed
exactly once) preceded by two dummy warm-up collectives that absorb the
~20us CC cold-start during the attention window; BN1 is folded into
conv2 (weights scaled by scl1, bias from host-precomputed tap-summed
weights, padding halo written as -sh1/scl1 so zero-padding is
reproduced exactly); output is stored bf16 and converted on host.

Device layout is channel-major ([C, spatial]); host does the (free)
transposes / weight reordering when staging inputs, and transposes the
per-core outputs back.
"""

import os
import sys

import numpy as np

for _p in ("/opt/trn_rl_repo", os.path.expanduser("~/.axon_site/_ro/trn_rl_repo")):
    if os.path.isdir(_p) and _p not in sys.path:
        sys.path.insert(0, _p)

import ml_dtypes

import concourse.bass as bass
import concourse.mybir as mybir
import concourse.tile as tile
from concourse import bacc
from concourse.bass_utils import run_bass_kernel_spmd

F32 = mybir.dt.float32
BF16 = mybir.dt.bfloat16
AF = mybir.ActivationFunctionType
ALU = mybir.AluOpType

NCORES = 8
H = W = 80
HW = H * W          # 6400
D = 256
NI = 3200           # inside positions (image rows 0..39)
NHEAD = 8
PW = W + 2          # 82 padded width
BN_EPS = 1e-5
BN_N = float(NCORES * HW)

# conv row-tiling: 5 output rows per psum tile -> N = 5*82 = 410 <= 512
RT = 5
NRT = H // RT       # 16
NT = RT * PW        # 410

LAST_EXEC_NS = None
LAST_MEAN_EXEC_NS = None

_cache = {}


def _bd(ap3):
    return ap3.rearrange("p a b -> p (a b)")


def _r3(ap2, a):
    return ap2.rearrange("p (a b) -> p a b", a=a)


def build_nc():
    nc = bacc.Bacc(
        "TRN2", target_bir_lowering=False, debug=False, num_devices=NCORES
    )

    ft_d = nc.dram_tensor("ft", [D, HW], F32, kind="ExternalInput")
    wqt_d = nc.dram_tensor("wqt", [128, 2, D], BF16, kind="ExternalInput")
    wkt_d = nc.dram_tensor("wkt", [128, 2, D], BF16, kind="ExternalInput")
    wvt_d = nc.dram_tensor("wvt", [128, 2, D], BF16, kind="ExternalInput")
    c1w_d = nc.dram_tensor("c1w", [128, 36, D], BF16, kind="ExternalInput")
    c2w_d = nc.dram_tensor("c2w", [128, 18, D], BF16, kind="ExternalInput")
    ws2_d = nc.dram_tensor("ws2", [128, 2, D], BF16, kind="ExternalInput")
    bn1g_d = nc.dram_tensor("bn1g", [D, 1], F32, kind="ExternalInput")
    bn1b_d = nc.dram_tensor("bn1b", [D, 1], F32, kind="ExternalInput")
    bn2g_d = nc.dram_tensor("bn2g", [D, 1], F32, kind="ExternalInput")
    bn2b_d = nc.dram_tensor("bn2b", [D, 1], F32, kind="ExternalInput")
    mblk_d = nc.dram_tensor("mblk", [8, 256], BF16, kind="ExternalInput")
    out_d = nc.dram_tensor("out_t", [D, HW], BF16, kind="ExternalOutput")

    groups = [list(range(NCORES))]

    with tile.TileContext(nc) as tc:
        with (
            tc.tile_pool(name="pers", bufs=1) as pers,
            tc.tile_pool(name="bigp", bufs=2) as bigp,
            tc.tile_pool(name="ftio", bufs=2) as ftio,
            tc.tile_pool(name="qtp", bufs=4) as qtp,
            tc.tile_pool(name="scr", bufs=3) as scr,
            tc.tile_pool(name="small", bufs=1) as small,
            tc.tile_pool(name="fin", bufs=3) as fin,
            tc.tile_pool(name="psA", bufs=4, space="PSUM") as psA,
            tc.tile_pool(name="psS", bufs=2, space="PSUM") as psS,
            tc.tile_pool(name="psC", bufs=2, space="PSUM") as psC,
            tc.tile_pool(name="dram", bufs=1, space="DRAM") as dramp,
        ):
            # ---------------- weights / consts (DMA: proj first, ft next) ---
            wqt = pers.tile([128, 2, D], BF16, tag="wqt", name="wqt")
            wkt = pers.tile([128, 2, D], BF16, tag="wkt", name="wkt")
            wvt = pers.tile([128, 2, D], BF16, tag="wvt", name="wvt")
            nc.sync.dma_start(wkt[:, :, :], wkt_d[:, :, :])
            nc.sync.dma_start(wvt[:, :, :], wvt_d[:, :, :])
            nc.sync.dma_start(wqt[:, :, :], wqt_d[:, :, :])

            # ft segments: (s, m) order so K/V projections can start early
            ftb = [
                pers.tile([128, HW], BF16, tag=f"ftb{m}", name=f"ftb{m}")
                for m in range(2)
            ]
            ip1 = [
                pers.tile([128, 84, PW], BF16, tag=f"ip1_{c}", name=f"ip1_{c}")
                for c in range(4)
            ]
            ip1f = [_bd(ip1[c][:, :, :]) for c in range(4)]

            # first-256-column slivers let the first K/V matmuls start
            # ~8us earlier than the full 1600-col segment loads allow
            ft_sl = {}
            for m in range(2):
                slv = ftio.tile([128, 256], F32, tag="sliv", name="sliv")
                nc.sync.dma_start(slv[:, :], ft_d[m * 128 : (m + 1) * 128, 0:256])
                ft_sl[m] = slv
            ft_stage = {}
            for s in range(4):
                for m in range(2):
                    lo = 256 if s == 0 else 0
                    ftt = ftio.tile([128, 1600 - lo], F32, tag="ftio", name="ftio")
                    eng = nc.sync if m == 0 else nc.gpsimd
                    eng.dma_start(
                        ftt[:, :],
                        ft_d[m * 128 : (m + 1) * 128,
                             s * 1600 + lo : (s + 1) * 1600],
                    )
                    ft_stage[(s, m)] = ftt

            c1w = pers.tile([128, 36, D], BF16, tag="c1w", name="c1w")
            c2w = pers.tile([128, 18, D], BF16, tag="c2w", name="c2w")
            ws2 = pers.tile([128, 2, D], BF16, tag="ws2", name="ws2")
            maskblk = pers.tile([8, 256], BF16, tag="maskblk", name="maskblk")
            nc.sync.dma_start(c1w[:, :, :], c1w_d[:, :, :])
            nc.sync.dma_start(c2w[:, :, :], c2w_d[:, :, :])
            nc.sync.dma_start(ws2[:, :, :], ws2_d[:, :, :])
            nc.sync.dma_start(maskblk[:, :], mblk_d[:, :])

            # warm up the CC cores with two dummy all-reduces: the first two
            # collective executions pay a ~20us cold-start which would
            # otherwise land in the BN1 bubble; here it overlaps attention.
            wrm = small.tile([8, 2], F32, tag="wrm", name="wrm")
            nc.vector.memset(wrm[:, :], 0.0)
            for w in range(2):
                win = dramp.tile([8, 2], F32, tag=f"win{w}", name=f"win{w}")
                wout = dramp.tile([8, 2], F32, tag=f"wout{w}", name=f"wout{w}")
                nc.sync.dma_start(win[:, :], wrm[:, :])
                nc.gpsimd.collective_compute(
                    "AllReduce", ALU.add, replica_groups=groups,
                    ins=[win[:, :].opt()], outs=[wout[:, :].opt()],
                )

            eps_t = small.tile([128, 1], F32, tag="eps_t", name="eps_t")
            nc.vector.memset(eps_t[:, :], BN_EPS)
            ones_t = small.tile([128, 164], F32, tag="ones_t", name="ones_t")
            nc.vector.memset(ones_t[:, :], 1.0)
            g1 = small.tile([128, 2], F32, tag="g1", name="g1")
            b1 = small.tile([128, 2], F32, tag="b1", name="b1")
            g2 = small.tile([128, 2], F32, tag="g2", name="g2")
            b2 = small.tile([128, 2], F32, tag="b2", name="b2")
            for o in range(2):
                sl = slice(o * 128, (o + 1) * 128)
                nc.sync.dma_start(g1[:, o : o + 1], bn1g_d[sl, :])
                nc.sync.dma_start(b1[:, o : o + 1], bn1b_d[sl, :])
                nc.sync.dma_start(g2[:, o : o + 1], bn2g_d[sl, :])
                nc.sync.dma_start(b2[:, o : o + 1], bn2b_d[sl, :])

            # ---------------- ip1 halo zeroing (targeted, not full tiles) ---
            # tile row r+2 == image row r; taps read tile rows 1..82 and the
            # 410-window spills touch rows 0 and 83 in discarded columns.
            for c in range(2):  # f-chunks
                nc.vector.memset(ip1[c][:, 0:2, :], 0.0)
                nc.vector.memset(ip1[c][:, 82:84, :], 0.0)
                nc.vector.memset(ip1[c][:, 2:82, 0:1], 0.0)
                nc.vector.memset(ip1[c][:, 2:82, 81:82], 0.0)
            for c in range(2, 4):  # t-chunks: zero band above message rows
                nc.vector.memset(ip1[c][:, 34:42, :], 0.0)
                nc.vector.memset(ip1[c][:, 82:84, :], 0.0)
                nc.vector.memset(ip1[c][:, 42:82, 0:1], 0.0)
                nc.vector.memset(ip1[c][:, 42:82, 81:82], 0.0)

            # ---------------- per-segment cast + ip1 interior + K/V proj ----
            ke = bigp.tile([128, 25, D], BF16, tag="big", name="ke")
            # ve layout [ones, v0..v255, ones]: per half m the 129 columns
            # [m*129 : m*129+129) are contiguous = [ones|v_m] or [v_m|ones]
            ve = bigp.tile([128, 25, D + 2], BF16, tag="big", name="ve")
            nc.vector.memset(ve[:, :, 0:1], 1.0)
            nc.vector.memset(ve[:, :, 257:258], 1.0)

            def kv_tile(i):
                ps = psA.tile([128, D], F32, tag="psA", name="psA")
                for ki in range(2):
                    nc.tensor.matmul(
                        ps[:, :],
                        ftb[ki][:, i * 128 : (i + 1) * 128],
                        wkt[:, ki, :],
                        start=(ki == 0),
                        stop=(ki == 1),
                    )
                # elu(x)+1 = relu(x) + exp(min(x,0))
                sm = scr.tile([128, 400], F32, tag="scr", name="sm")
                se = scr.tile([128, 400], F32, tag="scr", name="se")
                nc.vector.tensor_scalar_min(sm[:, :D], ps[:, :], 0.0)
                nc.scalar.activation(se[:, :D], sm[:, :D], AF.Exp)
                nc.vector.scalar_tensor_tensor(
                    ke[:, i, :], ps[:, :], 0.0, se[:, :D], ALU.max, ALU.add
                )
                ps2 = psA.tile([128, D], F32, tag="psA", name="psA")
                for ki in range(2):
                    nc.tensor.matmul(
                        ps2[:, :],
                        ftb[ki][:, i * 128 : (i + 1) * 128],
                        wvt[:, ki, :],
                        start=(ki == 0),
                        stop=(ki == 1),
                    )
                nc.vector.tensor_copy(ve[:, i, 1:257], ps2[:, :])

            def stage_seg(s, m):
                ftt = ft_stage[(s, m)]
                lo = 256 if s == 0 else 0
                nc.scalar.copy(
                    ftb[m][:, s * 1600 + lo : (s + 1) * 1600], ftt[:, :]
                )
                fseg = ftb[m][:, s * 1600 : (s + 1) * 1600]
                nc.vector.tensor_copy(
                    ip1[m][:, 2 + 20 * s : 22 + 20 * s, 1:81], _r3(fseg, 20)
                )

            for m in range(2):
                nc.scalar.copy(ftb[m][:, 0:256], ft_sl[m][:, :])
            kv_tile(0)
            kv_tile(1)
            for s in range(2):  # inside segments
                for m in range(2):
                    stage_seg(s, m)
                for i in range(12 * s + 2 * (1 - s), 12 * s + 12 + s):
                    kv_tile(i)  # s=0: 2..11, s=1: 12..24

            for s in range(2, 4):  # outside segments
                for m in range(2):
                    stage_seg(s, m)

            # ---------------- conv1 helpers ----------------
            y1p = [
                pers.tile([128, 84, PW], BF16, tag=f"y1p_{o}", name=f"y1p_{o}")
                for o in range(2)
            ]
            for o in range(2):  # spill-read guard rows (discarded columns)
                nc.vector.memset(y1p[o][:, 0:1, :], 0.0)
                nc.vector.memset(y1p[o][:, 83:84, :], 0.0)
            stats1 = small.tile([128, 4 * NRT], F32, tag="stats1", name="stats1")

            def conv1_tile(j):
                r0 = RT * j
                taps = []
                for c in range(4):
                    for ky in range(3):
                        if c >= 2 and r0 + ky + 4 < 41:
                            continue  # t-channel rows all zero
                        for kx in range(3):
                            taps.append((c, ky, kx))
                for o in range(2):
                    ps = psC.tile([128, NT], F32, tag="psC", name="psC")
                    for idx, (c, ky, kx) in enumerate(taps):
                        s = (r0 + ky + 1) * PW + kx - 1
                        nc.tensor.matmul(
                            ps[:, :],
                            c1w[:, (ky * 3 + kx) * 4 + c, o * 128 : (o + 1) * 128],
                            ip1f[c][:, s : s + NT],
                            start=(idx == 0),
                            stop=(idx == len(taps) - 1),
                        )
                    val = _r3(ps[:, :], RT)[:, :, 1:81]
                    nc.scalar.copy(
                        y1p[o][:, 2 + r0 : 7 + r0, 1:81], val
                    )
                    nc.vector.tensor_reduce(
                        stats1[:, (o * 2) * NRT + j : (o * 2) * NRT + j + 1],
                        val, mybir.AxisListType.XY, ALU.add,
                    )
                    sq = scr.tile([128, 400], F32, tag="scr", name="sq")
                    nc.scalar.activation(
                        _r3(sq[:, :], RT), val, AF.Square,
                        accum_out=stats1[:, (o * 2 + 1) * NRT + j :
                                         (o * 2 + 1) * NRT + j + 1],
                    )

            # f-only conv1 tiles 0,1 ahead of the KV->bd accumulation:
            # the bd chain waits on per-i scalar ve copies, so give the PE
            # filler work to absorb that lag.
            conv1_tile(0)
            conv1_tile(1)

            # ---------------- KV + Ksum -> block-diag BD ----------------
            bd = [
                pers.tile([128, 136], BF16, tag=f"bd{m}", name=f"bd{m}")
                for m in range(2)
            ]
            for m in range(2):
                psm = psA.tile([128, 129], F32, tag="psA", name="psA")
                for i in range(25):
                    nc.tensor.matmul(
                        psm[:, :],
                        ke[:, i, m * 128 : (m + 1) * 128],
                        ve[:, i, m * 129 : m * 129 + 129],
                        start=(i == 0),
                        stop=(i == 24),
                    )
                kcol = 0 if m == 0 else 128
                voff = 1 - m
                nc.vector.memset(bd[m][:, :], 0.0)
                for hh in range(4):
                    h = m * 4 + hh
                    lh = hh * 32
                    nc.vector.tensor_copy(
                        bd[m][lh : lh + 32, lh : lh + 32],
                        psm[lh : lh + 32, voff + lh : voff + lh + 32],
                    )
                    nc.vector.tensor_copy(
                        bd[m][lh : lh + 32, 128 + h : 129 + h],
                        psm[lh : lh + 32, kcol : kcol + 1],
                    )

            # ---------- Q/message pipeline interleaved with f-only conv1 ----
            def qproj(j):
                lsl = slice(NI + j * 400, NI + (j + 1) * 400)
                qt = [
                    qtp.tile([128, 400], BF16, tag="qteT", name=f"qt{m}")
                    for m in range(2)
                ]
                qps = []
                for m in range(2):
                    ps = psA.tile([128, 400], F32, tag="psA", name="psA")
                    for ki in range(2):
                        nc.tensor.matmul(
                            ps[:, :],
                            wqt[:, ki, m * 128 : (m + 1) * 128],
                            ftb[ki][:, lsl],
                            start=(ki == 0),
                            stop=(ki == 1),
                        )
                    qps.append(ps)
                return qt, qps

            def qelu(qt, qps):
                for m in range(2):
                    sm = scr.tile([128, 400], F32, tag="scr", name="smq")
                    se = scr.tile([128, 400], F32, tag="scr", name="seq")
                    nc.vector.tensor_scalar_min(sm[:, :], qps[m][:, :], 0.0)
                    nc.scalar.activation(se[:, :], sm[:, :], AF.Exp)
                    nc.vector.scalar_tensor_tensor(
                        qt[m][:, :], qps[m][:, :], 0.0, se[:, :], ALU.max, ALU.add
                    )

            def qmessage(j, qt):
                pss = psS.tile([8, 400], F32, tag="psS", name="psS")
                for ki in range(2):
                    nc.tensor.matmul(
                        pss[:, :],
                        bd[ki][:, 128:136],
                        qt[ki][:, :],
                        start=(ki == 0),
                        stop=(ki == 1),
                    )
                # S >> eps (S >= ~1e2), so 1/(S+eps) == 1/S in fp32
                rsf = scr.tile([128, 400], F32, tag="scr", name="rsf")
                rs = scr.tile([128, 400], BF16, tag="scr", name="rs")
                nc.vector.reciprocal_approx_fast(rsf[:8, :], pss[:, :])
                nc.scalar.copy(rs[:8, :], rsf[:8, :])

                for m in range(2):
                    psg = psA.tile([128, 400], F32, tag="psA", name="psA")
                    nc.tensor.matmul(
                        psg[:, :], bd[m][:, 0:128], qt[m][:, :],
                        start=True, stop=True,
                    )
                    pre = psS.tile([128, 400], F32, tag="psS", name="psS")
                    nc.tensor.matmul(
                        pre[:, :], maskblk[:, m * 128 : (m + 1) * 128], rs[:8, :]
                    )
                    preb = scr.tile([128, 400], BF16, tag="scr", name="preb")
                    nc.scalar.copy(preb[:, :], pre[:, :])
                    # l-tile j = image rows 40+5j..44+5j -> tile rows 42+5j..
                    nc.vector.tensor_tensor(
                        ip1[2 + m][:, 42 + 5 * j : 47 + 5 * j, 1:81],
                        _r3(psg[:, :], RT),
                        _r3(preb[:, :], RT),
                        ALU.mult,
                    )

            # software pipeline: Qproj j+1 is issued before the dependent
            # attention tail of j; f-only conv1 tiles fill the PE while the
            # vector engine runs elu/Z.
            qt_c, qps_c = qproj(0)
            for j in range(8):
                qelu(qt_c, qps_c)
                if j <= 4:
                    conv1_tile(j + 2)   # f-only tiles (t rows all zero)
                nxt = qproj(j + 1) if j < 7 else None
                qmessage(j, qt_c)
                if j >= 1:
                    # tile j+6 needs message j-1 (just written): keeps the PE
                    # queue deep so the elu chain latency never starves it
                    conv1_tile(j + 6)
                if nxt is not None:
                    qt_c, qps_c = nxt

            # pre-warm the sqrt activation table now that the last Exp is
            # issued: the 1.28us table switch hides under conv1 instead of
            # landing in the BN1 critical path.
            sqwrm = small.tile([128, 1], F32, tag="sqwrm", name="sqwrm")
            nc.scalar.activation(sqwrm[:, :], eps_t[:, :], AF.Sqrt)

            # two-chunk BN all-reduce: chunk A (tiles 0..CHA-1) is launched
            # while the PE is still crunching the remaining tiles; only the
            # small chunk-B collective latency is exposed.
            def ar_chunk(stats, lo, hi, tag):
                sv = stats[:, :].rearrange("p (k j) -> p k j", j=NRT)
                bnst = small.tile([128, 4], F32, tag=f"bnst{tag}", name=f"bnst{tag}")
                arin = dramp.tile([128, 4], F32, tag=f"arin{tag}", name=f"arin{tag}")
                arout = dramp.tile([128, 4], F32, tag=f"arout{tag}", name=f"arout{tag}")
                nc.vector.tensor_reduce(
                    bnst[:, :], sv[:, :, lo:hi], mybir.AxisListType.X, ALU.add
                )
                return bnst, arin, arout

            def ar_launch(bnst, arin, arout):
                nc.gpsimd.dma_start(arin[:, :], bnst[:, :])
                nc.gpsimd.collective_compute(
                    "AllReduce", ALU.add, replica_groups=groups,
                    ins=[arin[:, :].opt()], outs=[arout[:, :].opt()],
                )

            def ar_fetch(arout, tag):
                g = small.tile([128, 4], F32, tag=f"gst{tag}", name=f"gst{tag}")
                nc.gpsimd.dma_start(g[:, :], arout[:, :])
                return g

            # a single all-reduce per BN: launch skew between cores is
            # absorbed exactly once (chunked/split all-reduces each absorb
            # the full skew again since their triggers are independent).
            for j in range(14, NRT):
                conv1_tile(j)
            ar1 = ar_chunk(stats1, 0, NRT, "1")
            ar_launch(*ar1)
            gst1 = ar_fetch(ar1[2], "1")

            def bn_coeffs(gst, gg, bb, tag):
                nm = small.tile([128, 2], F32, tag=f"nm{tag}", name=f"nm{tag}")
                ex2 = small.tile([128, 2], F32, tag=f"ex2{tag}", name=f"ex2{tag}")
                var = small.tile([128, 2], F32, tag=f"var{tag}", name=f"var{tag}")
                sd = small.tile([128, 2], F32, tag=f"sd{tag}", name=f"sd{tag}")
                rsd = small.tile([128, 2], F32, tag=f"rsd{tag}", name=f"rsd{tag}")
                scl = small.tile([128, 2], F32, tag=f"scl{tag}", name=f"scl{tag}")
                sh = small.tile([128, 2], F32, tag=f"sh{tag}", name=f"sh{tag}")
                gv = gst[:, :].rearrange("p (o k) -> p k o", k=2)
                nc.vector.tensor_scalar_mul(nm[:, :], gv[:, 0, :], -1.0 / BN_N)
                nc.vector.tensor_scalar_mul(ex2[:, :], gv[:, 1, :], 1.0 / BN_N)
                # var_neg = m^2 - E[x^2];  sd = sqrt(-var_neg + eps)
                nc.vector.tensor_tensor(var[:, :], nm[:, :], nm[:, :], ALU.mult)
                nc.vector.tensor_tensor(
                    var[:, :], var[:, :], ex2[:, :], ALU.subtract
                )
                nc.scalar.activation(
                    sd[:, :], var[:, :], AF.Sqrt, bias=eps_t[:, 0:1], scale=-1.0
                )
                nc.vector.reciprocal(rsd[:, :], sd[:, :])
                nc.vector.tensor_tensor(scl[:, :], rsd[:, :], gg[:, :], ALU.mult)
                nc.vector.tensor_tensor(sh[:, :], nm[:, :], scl[:, :], ALU.mult)
                nc.vector.tensor_tensor(sh[:, :], sh[:, :], bb[:, :], ALU.add)
                return scl, sh

            scl1, sh1 = bn_coeffs(gst1, g1, b1, "1")

            # fold BN1 into conv2: w2' = w2 * scl1[c]; halo = -sh1/scl1 so
            # zero-padding maps to BN-output zero; bias2[o] = sum_{c,k} w2*sh1
            c2wv = c2w[:, :, :].rearrange("p (t c) o -> p t c o", c=2)
            for ck in range(2):
                nc.vector.tensor_scalar(
                    c2wv[:, :, ck, :], c2wv[:, :, ck, :],
                    scl1[:, ck : ck + 1], None, ALU.mult,
                )
            hv1 = small.tile([128, 2], F32, tag="hv1", name="hv1")
            rscl = small.tile([128, 2], F32, tag="rscl", name="rscl")
            nc.vector.reciprocal(rscl[:, :], scl1[:, :])
            nc.vector.scalar_tensor_tensor(
                hv1[:, :], sh1[:, :], -1.0, rscl[:, :], ALU.mult, ALU.mult
            )
            for o in range(2):
                hvo = hv1[:, o : o + 1]
                nc.vector.tensor_scalar(
                    y1p[o][:, 1:2, :], _r3(ones_t[:, 0:82], 1), hvo, None, ALU.mult
                )
                nc.vector.tensor_scalar(
                    y1p[o][:, 82:83, :], _r3(ones_t[:, 0:82], 1), hvo, None, ALU.mult
                )
                nc.vector.tensor_scalar(
                    y1p[o][:, 2:82, 0:1], _r3(ones_t[:, 0:80], 80), hvo, None,
                    ALU.mult,
                )
                nc.vector.tensor_scalar(
                    y1p[o][:, 2:82, 81:82], _r3(ones_t[:, 0:80], 80), hvo, None,
                    ALU.mult,
                )

            bias2 = small.tile([128, 2], F32, tag="bias2", name="bias2")
            sh1b = small.tile([128, 2], BF16, tag="sh1b", name="sh1b")
            nc.scalar.copy(sh1b[:, :], sh1[:, :])
            for o in range(2):
                psb = psC.tile([128, NT], F32, tag="psC", name="psC")
                for ck in range(2):
                    nc.tensor.matmul(
                        psb[:, 0:1],
                        ws2[:, ck, o * 128 : (o + 1) * 128],
                        sh1b[:, ck : ck + 1],
                        start=(ck == 0),
                        stop=(ck == 1),
                    )
                nc.scalar.copy(bias2[:, o : o + 1], psb[:, 0:1])

            # ---------------- conv2 (+ stats) ----------------
            y2 = [
                bigp.tile([128, HW], BF16, tag="big", name=f"y2_{o}")
                for o in range(2)
            ]
            stats2 = small.tile([128, 4 * NRT], F32, tag="stats2", name="stats2")
            y1pf = [_bd(y1p[c][:, :, :]) for c in range(2)]
            for j in range(NRT):
                r0 = RT * j
                for o in range(2):
                    ps = psC.tile([128, NT], F32, tag="psC", name="psC")
                    idx = 0
                    for c in range(2):
                        for ky in range(3):
                            for kx in range(3):
                                s = (r0 + ky + 1) * PW + kx - 1
                                nc.tensor.matmul(
                                    ps[:, :],
                                    c2w[:, (ky * 3 + kx) * 2 + c,
                                        o * 128 : (o + 1) * 128],
                                    y1pf[c][:, s : s + NT],
                                    start=(idx == 0),
                                    stop=(idx == 17),
                                )
                                idx += 1
                    val = _r3(ps[:, :], RT)[:, :, 1:81]
                    # y2 = conv2(BN1(y1)) = ps + bias2 (scalar adds the bias)
                    nc.scalar.activation(
                        _r3(y2[o][:, j * 400 : (j + 1) * 400], RT), val,
                        AF.Identity, bias=bias2[:, o : o + 1],
                    )
                    nc.vector.tensor_reduce(
                        stats2[:, (o * 2) * NRT + j : (o * 2) * NRT + j + 1],
                        val, mybir.AxisListType.XY, ALU.add,
                    )
                    sq = scr.tile([128, 400], F32, tag="scr", name="sq2")
                    nc.scalar.activation(
                        _r3(sq[:, :], RT), val, AF.Square,
                        bias=bias2[:, o : o + 1],
                        accum_out=stats2[:, (o * 2 + 1) * NRT + j :
                                         (o * 2 + 1) * NRT + j + 1],
                    )

            # ---------------- BN2 allreduce ----------------
            ar2 = ar_chunk(stats2, 0, NRT, "2")
            # reduce() summed raw psum values; the true sums need +HW*bias2
            # exactly once per core (the sq stats were already biased).
            bnst2 = ar2[0]
            for o in range(2):
                nc.vector.scalar_tensor_tensor(
                    bnst2[:, 2 * o : 2 * o + 1], bias2[:, o : o + 1],
                    float(HW), bnst2[:, 2 * o : 2 * o + 1], ALU.mult, ALU.add
                )
            ar_launch(*ar2)
            gst2 = ar_fetch(ar2[2], "2")
            scl2, sh2 = bn_coeffs(gst2, g2, b2, "2")

            # ---------------- BN2 + residual + store (bf16) ----------------
            for o in range(2):
                for j in range(4):
                    fsl = slice(1600 * j, 1600 * (j + 1))
                    tmp = fin.tile([128, 1600], BF16, tag="tmp", name="tmp")
                    nc.scalar.activation(
                        tmp[:, :], y2[o][:, fsl], AF.Identity,
                        bias=sh2[:, o : o + 1], scale=scl2[:, o : o + 1],
                    )
                    ost = fin.tile([128, 1600], BF16, tag="ost", name="ost")
                    nc.vector.tensor_tensor(
                        ost[:, :], tmp[:, :], ftb[o][:, fsl], ALU.add
                    )
                    oeng = nc.sync if j % 2 == 0 else nc.gpsimd
                    oeng.dma_start(out_d[o * 128 : (o + 1) * 128, fsl], ost[:, :])

    nc.compile()
    return nc


def _mblk():
    mb = np.zeros((8, 256), np.float32)
    for h in range(8):
        mb[h, h * 32 : (h + 1) * 32] = 1.0
    return mb.astype(ml_dtypes.bfloat16)


def _prep_inputs(feat0, zone_mask, w_q, w_k, w_v, conv1_w, bn1_g, bn1_b,
                 conv2_w, bn2_g, bn2_b, num_inside):
    B = feat0.shape[0]
    pos = np.asarray(zone_mask[:, :, 0])
    order = np.argsort(~pos, axis=1, kind="stable")
    assert np.array_equal(
        order[:, :num_inside],
        np.broadcast_to(np.arange(num_inside), (B, num_inside)),
    ), "kernel assumes inside positions are the first num_inside rows"
    assert num_inside == NI

    bf = ml_dtypes.bfloat16
    f32 = np.float32

    def wt(w):  # [dout, din] -> [128, 2, dout]: [p, ki, o] = w[o, ki*128+p]
        return np.ascontiguousarray(
            w.T.reshape(2, 128, D).transpose(1, 0, 2)
        ).astype(bf)

    def cw(w, nchunk):  # [O, I, 3, 3] -> [128, 9*nchunk, O]
        o_, i_, _, _ = w.shape
        r = w.transpose(2, 3, 1, 0).reshape(9, nchunk, 128, o_)
        return np.ascontiguousarray(
            r.transpose(2, 0, 1, 3).reshape(128, 9 * nchunk, o_)
        ).astype(bf)

    c2 = np.asarray(conv2_w, f32)
    # tap-summed conv2 weights for the folded-BN bias: [128, chunk, O]
    ws2 = np.ascontiguousarray(
        c2.sum(axis=(2, 3)).T.reshape(2, 128, D).transpose(1, 0, 2)
    ).astype(bf)

    common = {
        "wqt": wt(np.asarray(w_q, f32)),
        "wkt": wt(np.asarray(w_k, f32)),
        "wvt": wt(np.asarray(w_v, f32)),
        "c1w": cw(np.asarray(conv1_w, f32), 4),
        "c2w": cw(c2, 2),
        "ws2": ws2,
        "bn1g": np.asarray(bn1_g, f32).reshape(D, 1),
        "bn1b": np.asarray(bn1_b, f32).reshape(D, 1),
        "bn2g": np.asarray(bn2_g, f32).reshape(D, 1),
        "bn2b": np.asarray(bn2_b, f32).reshape(D, 1),
        "mblk": _mblk(),
    }
    in_maps = []
    for b in range(NCORES):
        m = dict(common)
        m["ft"] = np.ascontiguousarray(np.asarray(feat0[b], f32).T)
        in_maps.append(m)
    return in_maps


def kernel(feat0, zone_mask, w_q, w_k, w_v, conv1_w, bn1_g, bn1_b,
           conv2_w, bn2_g, bn2_b, H=80, W=80, B=8, D=256, num_inside=3200,
           **_ignored):
    global LAST_EXEC_NS, LAST_MEAN_EXEC_NS
    if "nc" not in _cache:
        _cache["nc"] = build_nc()
    nc = _cache["nc"]

    in_maps = _prep_inputs(feat0, zone_mask, w_q, w_k, w_v, conv1_w, bn1_g,
                           bn1_b, conv2_w, bn2_g, bn2_b, int(num_inside))
    trace = os.environ.get("KERNEL_TRACE", "0") == "1"
    res = run_bass_kernel_spmd(nc, in_maps, list(range(NCORES)), trace=trace)
    LAST_EXEC_NS = res.exec_time_ns
    LAST_MEAN_EXEC_NS = res.mean_exec_time_ns
    out = np.empty((NCORES, HW, 256), np.float32)
    for b in range(NCORES):
        out[b] = np.asarray(res.results[b]["out_t"], np.float32).T
    return out
